# revision 26
# baseline (speedup 1.0000x reference)
"""Trainium2 Bass kernel for nn_AttentionBlock (dense transformer block).

Sharding: 8 cores = (2 batches x 4 seq-chunks of 512 tokens).
Each core: rmsnorm + QKV + rope for its 512-token chunk, AllGather of K/V
(bf16) across its batch's 4 cores, full attention for its queries over all
16 heads, then wo + residual + rmsnorm + SwiGLU FFN for its rows.
Activations are feature-major on chip; host transposes I/O chunks.
Matmul dtypes: bf16 for QKV/scores/PV/wo, float32r for the FFN.
"""

from contextlib import ExitStack

import numpy as np
import ml_dtypes

import concourse.bacc as bacc
import concourse.mybir as mybir
import concourse.tile as tile
from concourse.bass_utils import run_bass_kernel_spmd

DIM = 1024
NHEAD = 16
HD = 64
SEQ = 2048
BSZ = 2
FFN = 2816
EPS = 1e-6
NCORES = 8
GROUP = 4             # cores per batch (sequence-sharding group)
CHUNK = SEQ // GROUP  # 512 local tokens per core
NKT = SEQ // 128      # 16 key tiles
NFT = DIM // 128      # 8 feature tiles
NMID = FFN // 128     # 22 ffn hidden tiles

FP32 = mybir.dt.float32
F32R = mybir.dt.float32r
BF16 = mybir.dt.bfloat16
AF = mybir.ActivationFunctionType
BF16_NP = ml_dtypes.bfloat16

_prog_cache = {}


def _build_program(use_mask: bool, apply_attn_w: bool, apply_ffn_w: bool,
                   debug: bool = False):
    nc = bacc.Bacc("TRN2", target_bir_lowering=False, debug=False,
                   num_devices=NCORES)

    # ---- DRAM I/O ----
    x_fm = nc.dram_tensor("x_fm", (DIM, CHUNK), FP32, kind="ExternalInput").ap()
    csa = nc.dram_tensor("csa", (128, CHUNK), FP32, kind="ExternalInput").ap()
    csb = nc.dram_tensor("csb", (128, CHUNK), FP32, kind="ExternalInput").ap()
    ones_col = nc.dram_tensor("ones_col", (128, 1), F32R, kind="ExternalInput").ap()
    ones_row = nc.dram_tensor("ones_row", (1, 128), FP32, kind="ExternalInput").ap()
    e2 = nc.dram_tensor("e2", (2, 128), FP32, kind="ExternalInput").ap()
    wqt = nc.dram_tensor("wqt", (DIM, DIM), BF16, kind="ExternalInput").ap()
    wkt = nc.dram_tensor("wkt", (DIM, DIM), BF16, kind="ExternalInput").ap()
    wvt = nc.dram_tensor("wvt", (DIM, DIM), BF16, kind="ExternalInput").ap()
    wot = nc.dram_tensor("wot", (DIM, DIM), BF16, kind="ExternalInput").ap()
    # w1t/w3t: pre-tiled [f, p, (k c)]; w2t: pre-tiled [m, p, (k c)] (host)
    w1t = nc.dram_tensor("w1t", (FFN, DIM), F32R, kind="ExternalInput").ap()
    w3t = nc.dram_tensor("w3t", (FFN, DIM), F32R, kind="ExternalInput").ap()
    w2t = nc.dram_tensor("w2t", (DIM, FFN), F32R, kind="ExternalInput").ap()
    if apply_attn_w:
        attnw = nc.dram_tensor("attnw", (DIM, 1), FP32, kind="ExternalInput").ap()
    if apply_ffn_w:
        ffnw = nc.dram_tensor("ffnw", (DIM, 1), FP32, kind="ExternalInput").ap()
    if use_mask:
        maskt = nc.dram_tensor("maskt", (SEQ, CHUNK), FP32, kind="ExternalInput").ap()
    out_fm = nc.dram_tensor("out_fm", (DIM, CHUNK), FP32, kind="ExternalOutput").ap()
    dbg = {}
    if debug:
        for nm, shape, ddt in [
                ("dbg_xn", (DIM, CHUNK), BF16),
                ("dbg_q", (DIM, CHUNK), BF16),
                ("dbg_kg", (GROUP * DIM, CHUNK), BF16),
                ("dbg_vg", (GROUP * CHUNK, DIM), BF16),
                ("dbg_p0", (128, NKT * 512), BF16),
                ("dbg_attn", (DIM, CHUNK), BF16),
                ("dbg_h", (DIM, CHUNK), FP32),
                ("dbg_hn", (DIM, CHUNK), FP32)]:
            dbg[nm] = nc.dram_tensor(nm, shape, ddt, kind="ExternalOutput").ap()

    groups = [list(range(GROUP)), list(range(GROUP, 2 * GROUP))]

    with tile.TileContext(nc) as tc, ExitStack() as ctx:
        # long-lived pool: constants + tiles that cross phase boundaries
        p0 = ctx.enter_context(tc.tile_pool(name="p0", bufs=1))
        dr = ctx.enter_context(tc.tile_pool(name="dr", bufs=1, space="DRAM"))

        cos_t = p0.tile([128, CHUNK], FP32, name="cos_t")
        sin_t = p0.tile([128, CHUNK], FP32, name="sin_t")
        onesc_t = p0.tile([128, 1], F32R, name="onesc_t")
        onesr_t = p0.tile([1, 128], FP32, name="onesr_t")
        e2_t = p0.tile([2, 128], FP32, name="e2_t")
        eps_t = p0.tile([1, 1], FP32, name="eps_t")
        nc.vector.memset(eps_t[:], EPS)
        nc.sync.dma_start(cos_t[:], csa[:])
        nc.sync.dma_start(sin_t[:], csb[:])
        nc.sync.dma_start(onesc_t[:], ones_col[:])
        nc.sync.dma_start(onesr_t[:], ones_row[:])
        nc.sync.dma_start(e2_t[:], e2[:])
        attnw_t = ffnw_t = None
        if apply_attn_w:
            attnw_t = p0.tile([128, NFT], FP32, name="attnw_t")
            nc.sync.dma_start(
                attnw_t[:], attnw.rearrange("(k p) o -> p (k o)", p=128))
        if apply_ffn_w:
            ffnw_t = p0.tile([128, NFT], FP32, name="ffnw_t")
            nc.sync.dma_start(
                ffnw_t[:], ffnw.rearrange("(k p) o -> p (k o)", p=128))

        # PE warm-up: a short dense bf16 matmul burst so HAM reaches 8/8
        # before the real pipeline starts
        warm_bf = p0.tile([128, CHUNK], BF16, name="warm_bf")
        nc.vector.tensor_copy(warm_bf[:], cos_t[:])
        with tc.tile_pool(name="psW", bufs=1, space="PSUM") as psW:
            wu_ps = psW.tile([128, 512], FP32, name="wu_ps", tag="wu")
            for it in range(14):
                nc.tensor.matmul(wu_ps[:], warm_bf[:, 0:128], warm_bf[:],
                                 start=(it == 0), stop=(it == 13))

        # hidden chunk (feature-major, kept for the attention residual)
        x_t = []
        for i in range(NFT):
            t = p0.tile([128, CHUNK], FP32, name=f"x_{i}", tag="x", bufs=NFT)
            nc.sync.dma_start(t[:], x_fm[i * 128:(i + 1) * 128, :])
            x_t.append(t)

        # wo weights: load early (p0), consumed in phase C
        wo_sb = []
        for k in range(NFT):
            t = p0.tile([128, DIM], BF16, name=f"wo_{k}", tag="wo", bufs=NFT)
            nc.sync.dma_start(t[:], wot[k * 128:(k + 1) * 128, :])
            wo_sb.append(t)

        def rmsnorm(sb, ps, src_tiles, out_dt, out_pool, w_tile, tagp):
            """src (8 fm fp32 tiles) -> normalized tiles of out_dt."""
            ss_ps = ps.tile([1, CHUNK], FP32, name=f"ss_{tagp}", tag="ss")
            for i in range(NFT):
                sq = sb.tile([128, CHUNK], F32R, name=f"sq_{tagp}_{i}",
                             tag="sq", bufs=3)
                nc.scalar.activation(sq[:], src_tiles[i][:], AF.Square)
                nc.tensor.matmul(ss_ps[:], onesc_t[:], sq[:],
                                 start=(i == 0), stop=(i == NFT - 1))
            # v = mean + eps ; r = rsqrt(v) with one Newton step
            v_t = sb.tile([1, CHUNK], FP32, name=f"v_{tagp}", tag="nv")
            nc.scalar.activation(v_t[:], ss_ps[:], AF.Copy,
                                 scale=1.0 / DIM, bias=EPS)
            sd_t = sb.tile([1, CHUNK], FP32, name=f"sd_{tagp}", tag="nsd")
            nc.scalar.activation(sd_t[:], ss_ps[:], AF.Sqrt,
                                 scale=1.0 / DIM, bias=eps_t[:])
            r0_t = sb.tile([1, CHUNK], FP32, name=f"r0_{tagp}", tag="nr0")
            nc.vector.reciprocal_approx_fast(r0_t[:], sd_t[:])
            t1 = sb.tile([1, CHUNK], FP32, name=f"t1_{tagp}", tag="nt1")
            nc.vector.tensor_mul(t1[:], r0_t[:], r0_t[:])
            nc.vector.tensor_mul(t1[:], t1[:], v_t[:])
            nc.vector.tensor_scalar(t1[:], t1[:], -0.5, 1.5,
                                    op0=mybir.AluOpType.mult,
                                    op1=mybir.AluOpType.add)
            r_t = sb.tile([1, CHUNK], FP32, name=f"r_{tagp}", tag="nr")
            nc.vector.tensor_mul(r_t[:], r0_t[:], t1[:])
            # broadcast r over 128 partitions via PE (fp32 matmul, K=1)
            rb_ps = ps.tile([128, CHUNK], FP32, name=f"rb_{tagp}", tag="nrb")
            nc.tensor.matmul(rb_ps[:], onesr_t[:], r_t[:], start=True, stop=True)
            outs = []
            for i in range(NFT):
                o = out_pool.tile([128, CHUNK], out_dt, name=f"n_{tagp}_{i}",
                                  tag=f"n_{tagp}", bufs=NFT)
                nc.vector.tensor_mul(o[:], src_tiles[i][:], rb_ps[:])
                if w_tile is not None:
                    nc.vector.tensor_scalar_mul(o[:], o[:], w_tile[:, i:i + 1])
                outs.append(o)
            return outs

        # q tiles (head-packed: [rA32 iA32 rB32 iB32] per tile) cross phases
        q_bf = [p0.tile([128, CHUNK], BF16, name=f"qbf_{m}", tag="qbf",
                        bufs=NFT) for m in range(NFT)]
        attn_fm = [p0.tile([128, CHUNK], BF16, name=f"attn_{hp}", tag="attn_fm",
                           bufs=NFT) for hp in range(NFT)]
        h_t = [p0.tile([128, CHUNK], FP32, name=f"h_{m}", tag="h", bufs=NFT)
               for m in range(NFT)]

        kg = dr.tile([GROUP * DIM, CHUNK], BF16, name="kg")
        vg = dr.tile([GROUP * CHUNK, DIM], BF16, name="vg")

        # tiny dummy AllGather: absorbs first-collective warmup cost while
        # phase A computes
        warm_in = dr.tile([128, 4], BF16, name="warm_in")
        warm_out = dr.tile([GROUP * 128, 4], BF16, name="warm_out")
        warm_sb = p0.tile([128, 4], BF16, name="warm_sb")
        nc.vector.memset(warm_sb[:], 0.0)
        nc.sync.dma_start(warm_in[:], warm_sb[:])
        nc.gpsimd.collective_compute(
            "AllGather", mybir.AluOpType.bypass, replica_groups=groups,
            ins=[warm_in.opt()], outs=[warm_out.opt()])

        # ================= phase A: rmsnorm1 + K -> AG, Q, V -> AG ========
        with tc.tile_pool(name="pA", bufs=1) as pA, \
             tc.tile_pool(name="psA", bufs=1, space="PSUM") as psA:

            xn = rmsnorm(pA, psA, x_t, BF16, pA,
                         attnw_t if apply_attn_w else None, "n1")

            def load_wmat(dram_ap, name, pool):
                tiles = []
                for k in range(NFT):
                    t = pool.tile([128, DIM], BF16, name=f"{name}_{k}",
                                  tag="wmat", bufs=2 * NFT)
                    nc.sync.dma_start(t[:], dram_ap[k * 128:(k + 1) * 128, :])
                    tiles.append(t)
                return tiles

            def rope_tensor(w_sb, outs, out_name, out_bufs):
                """Weight rows are [r-tiles 0..3 | i-tiles 0..3] (4 heads per
                tile, 32 rows each).  Produces head-packed tiles
                [rA32 iA32 rB32 iB32] for K=64 scores matmuls."""
                res = [None] * NFT
                for g in range(NFT // 2):
                    r_ps = psA.tile([128, CHUNK], FP32, name=f"{out_name}rps_{g}",
                                    tag="qkv_ps", bufs=3)
                    i_ps = psA.tile([128, CHUNK], FP32, name=f"{out_name}ips_{g}",
                                    tag="qkv_ps", bufs=3)
                    for k in range(NFT):
                        nc.tensor.matmul(r_ps[:],
                                         w_sb[k][:, g * 128:(g + 1) * 128],
                                         xn[k][:],
                                         start=(k == 0), stop=(k == NFT - 1))
                    for k in range(NFT):
                        nc.tensor.matmul(i_ps[:],
                                         w_sb[k][:, 512 + g * 128:512 + (g + 1) * 128],
                                         xn[k][:],
                                         start=(k == 0), stop=(k == NFT - 1))
                    t1 = pA.tile([128, CHUNK], BF16, name=f"{out_name}t1_{g}",
                                 tag="rope_t1", bufs=2)
                    t2 = pA.tile([128, CHUNK], BF16, name=f"{out_name}t2_{g}",
                                 tag="rope_t2", bufs=2)
                    t3 = pA.tile([128, CHUNK], BF16, name=f"{out_name}t3_{g}",
                                 tag="rope_t3", bufs=2)
                    t4 = pA.tile([128, CHUNK], BF16, name=f"{out_name}t4_{g}",
                                 tag="rope_t4", bufs=2)
                    nc.vector.tensor_mul(t1[:], r_ps[:], cos_t[:])
                    nc.vector.tensor_mul(t2[:], i_ps[:], sin_t[:])
                    nc.vector.tensor_mul(t3[:], r_ps[:], sin_t[:])
                    nc.vector.tensor_mul(t4[:], i_ps[:], cos_t[:])
                    for u in range(2):
                        hp = 2 * g + u
                        if outs is not None:
                            o = outs[hp]
                        else:
                            o = pA.tile([128, CHUNK], BF16,
                                        name=f"{out_name}_{hp}",
                                        tag=out_name, bufs=out_bufs)
                        for w in range(2):      # head within the pair
                            s0 = (2 * u + w) * 32
                            d0 = w * 64
                            nc.gpsimd.tensor_sub(o[d0:d0 + 32, :],
                                                 t1[s0:s0 + 32, :],
                                                 t2[s0:s0 + 32, :])
                            nc.gpsimd.tensor_add(o[d0 + 32:d0 + 64, :],
                                                 t3[s0:s0 + 32, :],
                                                 t4[s0:s0 + 32, :])
                        res[hp] = o
                return res

            # K first so the AllGather starts as early as possible
            wk_sb = load_wmat(wkt, "wk", pA)
            k_bf = rope_tensor(wk_sb, None, "kbf", 4)
            bounce_k = dr.tile([DIM, CHUNK], BF16, name="bounce_k")
            for m in range(NFT):
                nc.sync.dma_start(bounce_k[m * 128:(m + 1) * 128, :], k_bf[m][:])
            nc.gpsimd.collective_compute(
                "AllGather", mybir.AluOpType.bypass, replica_groups=groups,
                ins=[bounce_k.opt()], outs=[kg.opt()])

            # V (token-major) + bounce + AllGather
            wv_sb = load_wmat(wvt, "wv", pA)
            bounce_v = dr.tile([CHUNK, DIM], BF16, name="bounce_v")
            for t4_ in range(CHUNK // 128):
                for n2 in range(2):
                    v_ps = psA.tile([128, 512], FP32, name=f"vps_{t4_}_{n2}",
                                    tag="qkv_ps", bufs=3)
                    for k in range(NFT):
                        nc.tensor.matmul(v_ps[:],
                                         xn[k][:, t4_ * 128:(t4_ + 1) * 128],
                                         wv_sb[k][:, n2 * 512:(n2 + 1) * 512],
                                         start=(k == 0), stop=(k == NFT - 1))
                    v_bf = pA.tile([128, 512], BF16, name=f"vbf_{t4_}_{n2}",
                                   tag="vbf", bufs=2)
                    nc.vector.tensor_copy(v_bf[:], v_ps[:])
                    nc.sync.dma_start(
                        bounce_v[t4_ * 128:(t4_ + 1) * 128,
                                 n2 * 512:(n2 + 1) * 512], v_bf[:])
            nc.gpsimd.collective_compute(
                "AllGather", mybir.AluOpType.bypass, replica_groups=groups,
                ins=[bounce_v.opt()], outs=[vg.opt()])

            wq_sb = load_wmat(wqt, "wq", pA)
            rope_tensor(wq_sb, q_bf, "qbf", NFT)

            if debug:
                for i in range(NFT):
                    nc.sync.dma_start(dbg["dbg_xn"][i * 128:(i + 1) * 128, :], xn[i][:])
                    nc.sync.dma_start(dbg["dbg_q"][i * 128:(i + 1) * 128, :], q_bf[i][:])

        if debug:
            nc.sync.dma_start(dbg["dbg_kg"][:], kg[:])
            nc.sync.dma_start(dbg["dbg_vg"][:], vg[:])
        kg_r = kg.rearrange("(r f) t -> r f t", f=DIM)
        vg_r = vg.rearrange("(kt p) f -> p kt f", p=128)

        # ================= phase B: attention (16 heads, 8 pairs) =========
        # PE queue discipline: scores(hp) | PV(hp-1) | scores(hp+1) | ... so
        # the in-order PE stream never waits on the softmax-normalize chain;
        # normalization happens in a batched tail on sums copied to SBUF.
        NROUND = 8   # rounds of 2 k-tiles each
        with tc.tile_pool(name="pB", bufs=1) as pB, \
             tc.tile_pool(name="psB", bufs=1, space="PSUM") as psB:
            khh_t, vaug_t, probs_t, un_t, s2_t = {}, {}, {}, {}, {}

            def load_khh(hp):
                t = pB.tile([128, SEQ], BF16, name=f"khh_{hp}", tag="khh",
                            bufs=2)
                for r in range(GROUP):
                    nc.sync.dma_start(t[:, r * CHUNK:(r + 1) * CHUNK],
                                      kg_r[r, hp * 128:(hp + 1) * 128, :])
                khh_t[hp] = t

            def load_vaug(h):
                vaug = pB.tile([128, NKT * 65], BF16, name=f"vaug_{h}",
                               tag="vaug", bufs=3)
                vr = vaug.rearrange("p (kt c) -> p kt c", c=65)
                nc.gpsimd.dma_start(vr[:, :, 0:64],
                                    vg_r[:, :, h * 64:(h + 1) * 64])
                nc.vector.memset(vr[:, :, 64:65], 1.0)
                vaug_t[h] = vr

            def emit_scores(hp):
                khh = khh_t[hp]
                probss = []
                for w in range(2):
                    h = hp * 2 + w
                    probss.append(pB.tile([128, NKT * 512], BF16,
                                          name=f"probs_{h}", tag="probs",
                                          bufs=4))
                probs_t[hp] = probss
                if use_mask:
                    mrt = maskt.rearrange("(kt p) t -> p kt t", p=128)
                for rnd in range(NROUND):
                    scs = [psB.tile([128, 1024], FP32,
                                    name=f"sc_{hp}_{w}_{rnd}",
                                    tag="sc_ps", bufs=3) for w in range(2)]
                    for j in range(2):
                        kt = rnd * 2 + j
                        for w in range(2):
                            b0 = w * 64
                            nc.tensor.matmul(
                                scs[w][:, j * 512:(j + 1) * 512],
                                khh[b0:b0 + 64, kt * 128:(kt + 1) * 128],
                                q_bf[hp][b0:b0 + 64, :],
                                start=True, stop=True)
                    for w in range(2):
                        h = hp * 2 + w
                        if use_mask:
                            mt = pB.tile([128, 1024], FP32,
                                         name=f"mt_{h}_{rnd}", tag="mt", bufs=2)
                            mt_r = mt.rearrange("p (j t) -> p j t", j=2)
                            for j in range(2):
                                kt = rnd * 2 + j
                                nc.sync.dma_start(mt_r[:, j, :],
                                                  mrt[:, rnd * 2 + j, :])
                            nc.vector.tensor_scalar_mul(scs[w][:], scs[w][:],
                                                        0.125)
                            nc.vector.tensor_add(scs[w][:], scs[w][:], mt[:])
                            nc.scalar.activation(
                                probs_t[hp][w][:, rnd * 1024:(rnd + 1) * 1024],
                                scs[w][:], AF.Exp)
                        else:
                            nc.scalar.activation(
                                probs_t[hp][w][:, rnd * 1024:(rnd + 1) * 1024],
                                scs[w][:], AF.Exp, scale=0.125)
                if debug and hp == 0:
                    nc.sync.dma_start(dbg["dbg_p0"][:], probss[0][:])

            def emit_pv(hp):
                probss = probs_t.pop(hp)
                vaugs = [vaug_t.pop(hp * 2), vaug_t.pop(hp * 2 + 1)]
                s2 = pB.tile([1, 2 * CHUNK], FP32, name=f"s2_{hp}",
                             tag="s2", bufs=3)
                uns = []
                for w in range(2):
                    pv_ps = psB.tile([65, CHUNK], FP32, name=f"pv_{hp}_{w}",
                                     tag="pv_ps", bufs=2)
                    for kt in range(NKT):
                        nc.tensor.matmul(pv_ps[:],
                                         vaugs[w][:, kt, :],
                                         probss[w][:, kt * 512:(kt + 1) * 512],
                                         start=(kt == 0), stop=(kt == NKT - 1))
                    un = pB.tile([64, CHUNK], BF16, name=f"un_{hp}_{w}",
                                 tag="un", bufs=6)
                    nc.vector.tensor_copy(un[:], pv_ps[0:64, :])
                    nc.vector.tensor_copy(s2[0:1, w * CHUNK:(w + 1) * CHUNK],
                                          pv_ps[64:65, :])
                    uns.append(un)
                un_t[hp] = uns
                s2_t[hp] = s2

            def emit_norm(hp):
                s2 = s2_t.pop(hp)
                r2p = pB.tile([1, 2 * CHUNK], FP32, name=f"r2p_{hp}",
                              tag="r2p", bufs=2)
                nc.vector.reciprocal_approx_fast(r2p[:], s2[:])
                r2v = pB.tile([2, CHUNK], FP32, name=f"r2v_{hp}",
                              tag="r2v", bufs=2)
                nc.sync.dma_start(
                    r2v[:], r2p.rearrange("o (j t) -> o j t", j=2))
                rb_ps = psB.tile([128, CHUNK], FP32, name=f"rbp_{hp}",
                                 tag="sc_ps", bufs=3)
                nc.tensor.matmul(rb_ps[:], e2_t[:], r2v[:],
                                 start=True, stop=True)
                rb0 = pB.tile([64, CHUNK], FP32, name=f"rb0_{hp}",
                              tag="rb0", bufs=2)
                rb1 = pB.tile([64, CHUNK], FP32, name=f"rb1_{hp}",
                              tag="rb1", bufs=2)
                nc.vector.tensor_copy(rb0[:], rb_ps[0:64, :])
                nc.vector.tensor_copy(rb1[:], rb_ps[64:128, :])
                af = attn_fm[hp]
                u0, u1 = un_t.pop(hp)
                nc.vector.tensor_mul(af[0:64, :], u0[:], rb0[:])
                nc.vector.tensor_mul(af[64:128, :], u1[:], rb1[:])

            load_khh(0)
            load_khh(1)
            for h in range(6):
                load_vaug(h)
            NP = NHEAD // 2
            for hp in range(NP):
                if hp + 2 < NP:
                    load_khh(hp + 2)
                for h in (hp * 2 + 6, hp * 2 + 7):
                    if h < NHEAD:
                        load_vaug(h)
                emit_scores(hp)
                if hp >= 1:
                    emit_pv(hp - 1)
                if hp >= 2:
                    emit_norm(hp - 2)
                if hp >= 2:
                    khh_t.pop(hp - 2, None)
            emit_norm(NP - 2)
            emit_pv(NP - 1)
            emit_norm(NP - 1)

        # ================= phase C: wo + residual + rmsnorm2 ==============
        with tc.tile_pool(name="pC", bufs=1) as pC, \
             tc.tile_pool(name="psC", bufs=1, space="PSUM") as psC:
            for m in range(NFT):
                wo_ps = psC.tile([128, CHUNK], FP32, name=f"wops_{m}",
                                 tag="wo_ps", bufs=3)
                for k in range(NFT):
                    nc.tensor.matmul(wo_ps[:],
                                     wo_sb[k][:, m * 128:(m + 1) * 128],
                                     attn_fm[k][:],
                                     start=(k == 0), stop=(k == NFT - 1))
                nc.vector.tensor_add(h_t[m][:], x_t[m][:], wo_ps[:])

            hn = rmsnorm(pC, psC, h_t, F32R, p0,
                         ffnw_t if apply_ffn_w else None, "n2")
            if debug:
                for i in range(NFT):
                    nc.sync.dma_start(dbg["dbg_attn"][i * 128:(i + 1) * 128, :], attn_fm[i][:])
                    nc.sync.dma_start(dbg["dbg_h"][i * 128:(i + 1) * 128, :], h_t[i][:])
                    nc.sync.dma_start(dbg["dbg_hn"][i * 128:(i + 1) * 128, :], hn[i][:].bitcast(FP32))

        # ================= phase D: SwiGLU FFN (float32r) =================
        with tc.tile_pool(name="pD", bufs=1) as pD, \
             tc.tile_pool(name="psD", bufs=1, space="PSUM") as psD:
            w13_tiles = []
            for f in range(NMID):
                w1f = pD.tile([128, DIM], F32R, name=f"w1f_{f}", tag="w13",
                              bufs=8)
                nc.sync.dma_start(w1f[:], w1t[f * 128:(f + 1) * 128, :])
                w3f = pD.tile([128, DIM], F32R, name=f"w3f_{f}", tag="w13",
                              bufs=8)
                nc.sync.dma_start(w3f[:], w3t[f * 128:(f + 1) * 128, :])
                w13_tiles.append((w1f, w3f))
            mid = []
            for f in range(NMID):
                w1f, w3f = w13_tiles[f]
                g_ps = psD.tile([128, CHUNK], FP32, name=f"gps_{f}",
                                tag="g_ps", bufs=2)
                for k in range(NFT):
                    nc.tensor.matmul(g_ps[:], w1f[:, k * 128:(k + 1) * 128],
                                     hn[k][:],
                                     start=(k == 0), stop=(k == NFT - 1))
                sg = pD.tile([128, CHUNK], F32R, name=f"sg_{f}", tag="sg",
                             bufs=2)
                nc.scalar.activation(sg[:], g_ps[:], AF.Silu)
                u_ps = psD.tile([128, CHUNK], FP32, name=f"ups_{f}",
                                tag="u_ps", bufs=2)
                for k in range(NFT):
                    nc.tensor.matmul(u_ps[:], w3f[:, k * 128:(k + 1) * 128],
                                     hn[k][:],
                                     start=(k == 0), stop=(k == NFT - 1))
                md = pD.tile([128, CHUNK], F32R, name=f"mid_{f}", tag="mid",
                             bufs=NMID)
                nc.vector.tensor_mul(md[:], sg[:], u_ps[:])
                mid.append(md)

            for m in range(NFT):
                w2m = pD.tile([128, NMID * 128], F32R, name=f"w2m_{m}",
                              tag="w2m", bufs=2)
                nc.sync.dma_start(w2m[:], w2t[m * 128:(m + 1) * 128, :])
                o_ps = psD.tile([128, CHUNK], FP32, name=f"ops_{m}",
                                tag="o_ps", bufs=2)
                for f in range(NMID):
                    nc.tensor.matmul(o_ps[:], w2m[:, f * 128:(f + 1) * 128],
                                     mid[f][:],
                                     start=(f == 0), stop=(f == NMID - 1))
                ot = pD.tile([128, CHUNK], FP32, name=f"ot_{m}", tag="ot",
                             bufs=2)
                nc.vector.tensor_add(ot[:], h_t[m][:], o_ps[:])
                nc.sync.dma_start(out_fm[m * 128:(m + 1) * 128, :], ot[:])

    nc.compile()
    return nc


def _get_program(use_mask, apply_attn_w, apply_ffn_w, debug=False):
    key = (use_mask, apply_attn_w, apply_ffn_w, debug)
    if key not in _prog_cache:
        _prog_cache[key] = _build_program(*key)
    return _prog_cache[key]


def _rope_perm():
    """Row permutation: real (even) features of all heads first (4 tiles of
    4 heads x 32), then imag (odd) features in the same head order."""
    r_idx = np.concatenate([h * HD + 2 * np.arange(32) for h in range(NHEAD)])
    i_idx = np.concatenate([h * HD + 1 + 2 * np.arange(32) for h in range(NHEAD)])
    return np.concatenate([r_idx, i_idx])


def _tile_w13(w):
    """w (FFN, DIM) -> pre-tiled (FFN, DIM): block f rows = SBUF tile
    [p, (k c)] with value w.T[k*128+p, f*128+c]."""
    a = np.asarray(w, np.float32).reshape(NMID, 128, NFT, 128)  # [f, c, k, p]
    return np.ascontiguousarray(
        a.transpose(0, 3, 2, 1).reshape(NMID * 128, NFT * 128))


def _tile_w2(w):
    """w (DIM, FFN) -> pre-tiled (DIM, FFN): block m rows = SBUF tile
    [p, (k c)] with value w.T[k*128+p, m*128+c]."""
    a = np.asarray(w, np.float32).reshape(NFT, 128, NMID, 128)  # [m, c, k, p]
    return np.ascontiguousarray(
        a.transpose(0, 3, 2, 1).reshape(NFT * 128, NMID * 128))


def _prepare(inputs):
    hidden = np.ascontiguousarray(np.asarray(inputs["hidden_states_in"], np.float32))
    cos = np.asarray(inputs["freqs_cos"], np.float32)
    sin = np.asarray(inputs["freqs_sin"], np.float32)
    mask = np.asarray(inputs["mask"], np.float32)
    attn_w = np.asarray(inputs["attn_norm_w"], np.float32)
    ffn_w = np.asarray(inputs["ffn_norm_w"], np.float32)
    start_pos = int(np.asarray(inputs["start_pos"]))
    assert start_pos == 0, f"kernel only supports start_pos=0, got {start_pos}"

    use_mask = bool(np.any(mask))
    apply_attn_w = not bool(np.all(attn_w == 1.0))
    apply_ffn_w = not bool(np.all(ffn_w == 1.0))

    perm = _rope_perm()
    wq = np.asarray(inputs["wq"], np.float32)[perm, :]
    wk = np.asarray(inputs["wk"], np.float32)[perm, :]
    wv = np.asarray(inputs["wv"], np.float32)
    wo = np.asarray(inputs["wo"], np.float32)
    shared = {
        "wqt": np.ascontiguousarray(wq.T).astype(BF16_NP),
        "wkt": np.ascontiguousarray(wk.T).astype(BF16_NP),
        "wvt": np.ascontiguousarray(wv.T).astype(BF16_NP),
        "wot": np.ascontiguousarray(wo.T).astype(BF16_NP),
        "w1t": _tile_w13(inputs["w1"]),
        "w3t": _tile_w13(inputs["w3"]),
        "w2t": _tile_w2(inputs["w2"]),
        "ones_col": np.ones((128, 1), np.float32),
        "ones_row": np.ones((1, 128), np.float32),
    }
    e2 = np.zeros((2, 128), np.float32)
    e2[0, 0:64] = 1.0
    e2[1, 64:128] = 1.0
    shared["e2"] = e2
    if apply_attn_w:
        shared["attnw"] = attn_w.reshape(DIM, 1)
    if apply_ffn_w:
        shared["ffnw"] = ffn_w.reshape(DIM, 1)

    in_maps = []
    for c in range(NCORES):
        b = c // GROUP
        s0 = (c % GROUP) * CHUNK
        m = dict(shared)
        m["x_fm"] = np.ascontiguousarray(hidden[b, s0:s0 + CHUNK, :].T)
        cc = np.ascontiguousarray(cos[s0:s0 + CHUNK, :].T)  # (32, CHUNK)
        ss = np.ascontiguousarray(sin[s0:s0 + CHUNK, :].T)
        m["csa"] = np.ascontiguousarray(np.tile(cc, (4, 1)))  # cos, 4 heads/tile
        m["csb"] = np.ascontiguousarray(np.tile(ss, (4, 1)))  # sin
        if use_mask:
            m["maskt"] = np.ascontiguousarray(mask[b, s0:s0 + CHUNK, :].T)
        in_maps.append(m)
    return in_maps, (use_mask, apply_attn_w, apply_ffn_w)


def _assemble(results):
    out = np.empty((BSZ, SEQ, DIM), np.float32)
    for c in range(NCORES):
        b = c // GROUP
        s0 = (c % GROUP) * CHUNK
        out[b, s0:s0 + CHUNK, :] = results[c]["out_fm"].T
    return out


def run(inputs, trace=False, debug=False):
    in_maps, key = _prepare(inputs)
    nc = _get_program(*key, debug=debug)
    res = run_bass_kernel_spmd(nc, in_maps, core_ids=list(range(NCORES)),
                               trace=trace)
    return _assemble(res.results), res


def kernel(**inputs) -> np.ndarray:
    out, _ = run(inputs)
    return out
